# revision 46
# baseline (speedup 1.0000x reference)
"""DeepSeek sparse attention on 8 Trainium2 NeuronCores (Bass/Tile).

Strategy (2 SPMD launches, head-parallel, f32r indexer + fp16 attention):

  host: fuse indexer weights through the attention projections:
      Wfq = Wq @ Wq_ind, Wfk = Wk @ Wk_ind  (so the indexer reads
      hidden directly and needs no q_lin/k_lin round trip).
  A   (fused projections + indexer): core c keeps hidden^T resident in
      SBUF and runs: wfq -> qp_c^T, wfk -> kp_c^T (indexer head c,
      f32r), then the relu(qp.kp) score reduction -> rel_c interleaved
      with the wq pass (the PE fills score-drain stalls with wq
      matmuls), then wk/wv passes.  qT/kT/vT (fp16) are the core's own
      2 attention heads.
  host: rel = sum_c w_c * rel_c; top-1024 keys -> selected mask ->
      hi[k] threshold vector (fp16).
  B   (attention, fp16): core c computes softmax attention for its 2
      heads, masked causal/local/selected; the output projection for
      query chunk qc is interleaved right after head 1's softmax chain
      for qc completes, so out-DMA overlaps compute.
  host: out = sum_c partial_c.

The indexer path stays f32r end-to-end: the top-k boundary gaps are
~1e-5 relative, and selection flips cost ~1e-2 output error each.
The attention path is fp16 (storage) with f32 PE accumulation.
"""

import math

import numpy as np

import concourse.bass as bass
import concourse.mybir as mybir
from concourse import bacc
from concourse.tile import TileContext
from concourse.masks import make_identity
from concourse.bass_utils import run_bass_kernel_spmd

# Problem constants (hardcoded per contract)
HIDDEN = 2048
NUM_HEADS = 16
HEAD_DIM = 128
NUM_IND_HEADS = 8
IND_DIM = HIDDEN // NUM_IND_HEADS  # 256
MAX_SELECTED = 1024
LOCAL_WINDOW = 512
N_CORES = 8

F32 = mybir.dt.float32
F32R = mybir.dt.float32r
F16 = mybir.dt.float16
F8 = mybir.dt.float8e4
FP32 = np.float32

import ml_dtypes

E4M3 = ml_dtypes.float8_e4m3fn
SH = 8.0    # fp8 hidden scale
SW = 512.0  # fp8 weight scale (descale 1/(SH*SW) folded into evictions)

_TRACE = {"on": False, "exec_ns": []}


def build_fused(S=2048, H=HIDDEN, CS=HIDDEN // N_CORES, D=IND_DIM):
    """Launch A: hidden^T resident; qp/kp passes, scores x wq interleave,
    then wk/wv passes."""
    nc = bacc.Bacc("TRN2", target_bir_lowering=False, debug=False)
    HT, NQ, QT, DC, MC = H // 128, S // 512, S // 128, D // 128, CS // 128
    # all inputs arrive partition-major (host pre-rearranged): x[p, t*C+c]
    # = orig[t*128+p, c] — plain 2D DMAs, 128 descriptors each.
    hidT = nc.dram_tensor("hidT", [128, HT * S], F32R, kind="ExternalInput")
    wq = nc.dram_tensor("wq", [128, HT * CS], F32R, kind="ExternalInput")
    wk = nc.dram_tensor("wk", [128, HT * CS], F32R, kind="ExternalInput")
    wv = nc.dram_tensor("wv", [128, HT * CS], F32R, kind="ExternalInput")
    wfq = nc.dram_tensor("wfq", [128, HT * D], F32R, kind="ExternalInput")
    wfk = nc.dram_tensor("wfk", [128, HT * D], F32R, kind="ExternalInput")
    qT = nc.dram_tensor("qT", [CS, S], F16, kind="ExternalOutput")
    kT = nc.dram_tensor("kT", [CS, S], F16, kind="ExternalOutput")
    vT = nc.dram_tensor("vT", [CS, S], F16, kind="ExternalOutput")
    # rel written as the raw [128, QT] relmat; host unshuffles (p,t)->t*128+p
    rel = nc.dram_tensor("rel", [128, QT], F32, kind="ExternalOutput")

    AF = mybir.ActivationFunctionType

    with TileContext(nc) as tc:
        with (
            tc.tile_pool(name="hid", bufs=1) as hpool,
            tc.tile_pool(name="wres", bufs=1) as wrpool,
            tc.tile_pool(name="wt", bufs=2) as wpool,
            tc.tile_pool(name="proj", bufs=1) as ppool,
            tc.tile_pool(name="ev", bufs=4) as opool,
            tc.tile_pool(name="scr", bufs=2) as scpool,
            tc.tile_pool(name="rc", bufs=2) as rcpool,
            tc.tile_pool(name="rm", bufs=1) as rmpool,
            tc.tile_pool(name="ps", bufs=1, space="PSUM") as pspool,
        ):
            # hidden^T resident, loaded strip by strip (1 MB each) so the
            # first matmuls only wait on strip 0.
            hidc = [None] * HT

            def load_hid(t):
                hc = hpool.tile([128, S], F32R, name=f"hidc{t}")
                nc.sync.dma_start(out=hc, in_=hidT[:, t * S:(t + 1) * S])
                hidc[t] = hc

            WG = 4  # weight strips per streamed DMA group

            def load_wgroup(wd, g):
                w = wpool.tile([128, WG * CS], F32R, tag="ws", name="ws")
                nc.sync.dma_start(
                    out=w, in_=wd[:, g * WG * CS:(g + 1) * WG * CS])
                return w

            # DMA issue order = consumption order: hid0, wfq g0, hid1-2,
            # wfq g1, hid3-5, g2, hid6-8, g3, hid9-15.  wfk prefetched
            # during the qp pass; wq (full 2 MB resident) during kp.
            load_hid(0)
            wfq_groups = [load_wgroup(wfq, 0), None, None, None]
            load_hid(1), load_hid(2)
            wfq_groups[1] = load_wgroup(wfq, 1)
            load_hid(3), load_hid(4), load_hid(5)
            wfq_groups[2] = load_wgroup(wfq, 2)
            load_hid(6), load_hid(7), load_hid(8)
            wfq_groups[3] = load_wgroup(wfq, 3)
            for t in range(9, HT):
                load_hid(t)

            def hstrip(t):
                return hidc[t]

            qpt = [ppool.tile([128, S], F32R, name=f"qpt{i}") for i in range(DC)]
            kpt = [ppool.tile([128, S], F32R, name=f"kpt{i}") for i in range(DC)]

            PTAGS = [f"m{i}" for i in range(4)] + [f"s{i}" for i in range(4)]

            def proj_pass(wgroups, wd, sink, prefetch=None):
                # full 8-bank pass over all (mc, qc); streamed weight groups
                psq = [
                    pspool.tile([128, 512], F32, tag=PTAGS[i], name=PTAGS[i])
                    for i in range(MC * NQ)
                ]
                for t in range(HT):
                    g, tl = t // WG, t % WG
                    if wgroups[g] is None:
                        wgroups[g] = load_wgroup(wd, g)
                    if tl == 0 and g + 1 < len(wgroups) and wgroups[g + 1] is None:
                        wgroups[g + 1] = load_wgroup(wd, g + 1)
                    if prefetch is not None and t == HT // 2:
                        prefetch()
                    w = wgroups[g][:, tl * CS:(tl + 1) * CS]
                    rhs = hstrip(t)
                    for mc in range(MC):
                        for qc in range(NQ):
                            nc.tensor.matmul(
                                psq[mc * NQ + qc],
                                w[:, mc * 128:(mc + 1) * 128],
                                rhs[:, qc * 512:(qc + 1) * 512],
                                start=(t == 0), stop=(t == HT - 1),
                            )
                for mc in range(MC):
                    for qc in range(NQ):
                        sink(mc, qc, psq[mc * NQ + qc])

            ecnt = [0]

            def copy3(out_ap, ps):
                # psum->sbuf copies alternate ACT/DVE (gpsimd cannot read
                # PSUM — neuronxcc rejects it)
                eng = (nc.scalar.copy, nc.vector.tensor_copy)[ecnt[0] % 2]
                ecnt[0] += 1
                eng(out_ap, ps)

            def sink_proj(dst):
                def s(mc, qc, ps):
                    copy3(dst[mc][:, qc * 512:(qc + 1) * 512], ps)
                return s

            def sink_out(odram):
                def s(mc, qc, ps):
                    ot = opool.tile([128, 512], F16, tag="ot", name="ot")
                    copy3(ot, ps)
                    nc.gpsimd.dma_start(
                        out=odram[mc * 128:(mc + 1) * 128,
                                  qc * 512:(qc + 1) * 512],
                        in_=ot,
                    )
                return s

            # --- qp pass; prefetch wfk groups 0/1 halfway through ---
            wfk_groups = [None, None, None, None]

            def pf_wfk():
                wfk_groups[0] = load_wgroup(wfk, 0)
                wfk_groups[1] = load_wgroup(wfk, 1)

            proj_pass(wfq_groups, wfq, sink_proj(qpt), prefetch=pf_wfk)

            # --- kp pass; prefetch resident wq (2 MB) halfway through ---
            wq_res = [None]

            def pf_wq():
                wr = wrpool.tile([128, HT * CS], F32R, tag="wres", name="wqr")
                nc.sync.dma_start(out=wr, in_=wq[:, :])
                wq_res[0] = wr

            proj_pass(wfk_groups, wfk, sink_proj(kpt), prefetch=pf_wq)

            # --- scores x wq interleave ---
            # scores: per qt, 8 matmuls into 4 banks (s0..s3); the relu
            # reduction drains on ACT (tiles 0/1) + DVE (tiles 2/3).  The
            # drain rate (~2.5us/qt) exceeds the PE's 1.8us/qt, so wq
            # matmul chains (single-bank, 16-deep accumulation per
            # (mc,qc)) are interleaved to keep the PE busy.
            relmat = rmpool.tile([128, QT], F32, name="relmat")

            def score_qt(qt):
                spss = [
                    pspool.tile([128, 512], F32, tag=f"s{kc}", name="sps")
                    for kc in range(NQ)
                ]
                for dcc in range(DC):
                    for kc in range(NQ):
                        nc.tensor.matmul(
                            spss[kc],
                            qpt[dcc][:, qt * 128:(qt + 1) * 128],
                            kpt[dcc][:, kc * 512:(kc + 1) * 512],
                            start=(dcc == 0), stop=(dcc == DC - 1),
                        )
                relcols = rcpool.tile([128, NQ + 1], F32, tag="relcols",
                                      name="relcols")
                for kc in (0, 1):
                    scratch = scpool.tile([128, 512], F16, tag=f"scr{kc}",
                                          name="scratch")
                    nc.scalar.activation(
                        scratch, spss[kc], AF.Relu,
                        accum_out=relcols[:, kc:kc + 1],
                    )
                wide = scpool.tile([128, 1024], F16, tag="scrw", name="scrw")
                nc.vector.tensor_scalar_max(wide[:, 0:512], spss[2], 0.0)
                nc.vector.tensor_scalar_max(wide[:, 512:1024], spss[3], 0.0)
                nc.vector.tensor_reduce(
                    relcols[:, 2:3], wide,
                    axis=mybir.AxisListType.X, op=mybir.AluOpType.add,
                )
                relscr = rcpool.tile([128, 3], F32, tag="relscr",
                                     name="relscr")
                nc.scalar.activation(
                    relscr, relcols[:, 0:3], AF.Copy,
                    accum_out=relmat[:, qt:qt + 1],
                )

            # wq interleave: 8 chains of (mc,qc), each 16 accumulating
            # matmuls into one rotating bank (m0..m3).
            wq_sink = sink_out(qT)
            wq_chains = [(mc, qc) for mc in range(MC) for qc in range(NQ)]
            wq_state = {"chain": 0, "t": 0, "ps": None}

            def wq_step(nmm):
                # emit nmm wq matmuls (advancing chain state)
                wr = wq_res[0]
                for _ in range(nmm):
                    ci = wq_state["chain"]
                    if ci >= len(wq_chains):
                        return
                    mc, qc = wq_chains[ci]
                    t = wq_state["t"]
                    if t == 0:
                        wq_state["ps"] = pspool.tile(
                            [128, 512], F32, tag=f"m{ci % 4}", name="wqps")
                    nc.tensor.matmul(
                        wq_state["ps"],
                        wr[:, t * CS + mc * 128:t * CS + mc * 128 + 128],
                        hstrip(t)[:, qc * 512:(qc + 1) * 512],
                        start=(t == 0), stop=(t == HT - 1),
                    )
                    if t == HT - 1:
                        wq_sink(mc, qc, wq_state["ps"])
                        wq_state["chain"] = ci + 1
                        wq_state["t"] = 0
                    else:
                        wq_state["t"] = t + 1

            wk_groups = [None, None, None, None]

            def pf_wk():
                wk_groups[0] = load_wgroup(wk, 0)
                wk_groups[1] = load_wgroup(wk, 1)

            for qt in range(QT):
                score_qt(qt)
                if qt == QT - 4:
                    pf_wk()
                wq_step(7)  # 16 qt x 7 ~= 128 posted; rest drained after
            wq_step(128)
            nc.sync.dma_start(out=rel[:, :], in_=relmat)

            # --- wk / wv full 8-bank passes ---
            wv_groups = [None, None, None, None]

            def pf_wv():
                wv_groups[0] = load_wgroup(wv, 0)
                wv_groups[1] = load_wgroup(wv, 1)

            proj_pass(wk_groups, wk, sink_out(kT), prefetch=pf_wv)
            proj_pass(wv_groups, wv, sink_out(vT))
    nc.compile()
    return nc


def build_attn(S=2048, H=HIDDEN, NHC=NUM_HEADS // N_CORES, HD=HEAD_DIM,
               window=LOCAL_WINDOW):
    """Launch B: per-core (2 heads) masked softmax attention + out-proj,
    with the out-proj interleaved into head 1's attention."""
    nc = bacc.Bacc("TRN2", target_bir_lowering=False, debug=False)
    KC, NQ, QT, OCC = S // 128, S // 512, S // 128, H // 512
    qTh = nc.dram_tensor("qTh", [NHC * HD, S], F16, kind="ExternalInput")
    kTh = nc.dram_tensor("kTh", [NHC * HD, S], F16, kind="ExternalInput")
    vTh = nc.dram_tensor("vTh", [NHC * HD, S], F16, kind="ExternalInput")
    woh = nc.dram_tensor("woh", [NHC * HD, H], F16, kind="ExternalInput")
    iotar = nc.dram_tensor("iotar", [128, S], F16, kind="ExternalInput")
    hivec = nc.dram_tensor("hivec", [S], F16, kind="ExternalInput")
    selv = nc.dram_tensor("selv", [S], F16, kind="ExternalInput")
    onesrow = nc.dram_tensor("onesrow", [128], F32R, kind="ExternalInput")
    part = nc.dram_tensor("part", [S, H], F16, kind="ExternalOutput")

    scale = 1.0 / math.sqrt(HD)
    AF = mybir.ActivationFunctionType
    OP = mybir.AluOpType
    VSL_KC = 8  # far blocks only exist for kc <= 7

    with TileContext(nc) as tc:
        with (
            tc.tile_pool(name="const", bufs=1) as cpool,
            tc.tile_pool(name="qk", bufs=1) as qkpool,
            tc.tile_pool(name="vt", bufs=1) as vtpool,
            tc.tile_pool(name="vh", bufs=1) as vhpool,
            tc.tile_pool(name="vsl", bufs=1) as vslpool,
            tc.tile_pool(name="et", bufs=3) as etpool,
            tc.tile_pool(name="aon", bufs=1) as aopool,
            tc.tile_pool(name="dr", bufs=2) as drpool,
            tc.tile_pool(name="ev", bufs=4) as evpool,
            tc.tile_pool(name="ps", bufs=1, space="PSUM") as pspool,
        ):
            # DMA priority: head-0 q/k first (gates first matmul), then
            # v0, consts, head-1 tensors, wo.
            qsb, ksb, vts = [None, None], [None, None], [None, None]
            for h in range(NHC):
                qsb[h] = qkpool.tile([128, S], F16, name=f"qsb{h}")
                nc.sync.dma_start(out=qsb[h], in_=qTh[h * HD:(h + 1) * HD, :])
                ksb[h] = qkpool.tile([128, S], F16, name=f"ksb{h}")
                nc.sync.dma_start(out=ksb[h], in_=kTh[h * HD:(h + 1) * HD, :])
                if h == 0:
                    vts[0] = vtpool.tile([128, S], F16, name="vts0")
                    nc.sync.dma_start(out=vts[0], in_=vTh[0:HD, :])
                    ident = cpool.tile([128, 128], F16, name="ident")
                    make_identity(nc, ident)
                    iota = cpool.tile([128, S], F16, name="iota")
                    nc.sync.dma_start(out=iota, in_=iotar[:, :])
                    ones = cpool.tile([128, 1], F16, name="ones")
                    nc.vector.memset(ones, 1.0)
                    hvec = cpool.tile([128, KC], F16, name="hvec")
                    nc.sync.dma_start(
                        out=hvec, in_=hivec.rearrange("(t p) -> p t", p=128))
                    svec = cpool.tile([128, KC], F16, name="svec")
                    nc.sync.dma_start(
                        out=svec, in_=selv.rearrange("(t p) -> p t", p=128))
                    svec32 = cpool.tile([128, KC], F32, name="svec32")
                    nc.vector.tensor_copy(svec32, svec)
                    ones1 = cpool.tile([1, 128], F32R, name="ones1")
                    nc.sync.dma_start(out=ones1, in_=onesrow[None, :])
            vts[1] = vtpool.tile([128, S], F16, name="vts1")
            nc.sync.dma_start(out=vts[1], in_=vTh[HD:2 * HD, :])
            wsb = []
            for h in range(NHC):
                w = qkpool.tile([128, H], F16, name=f"wsb{h}")
                nc.sync.dma_start(out=w, in_=woh[h * HD:(h + 1) * HD, :])
                wsb.append(w)

            aon = [aopool.tile([128, S], F16, name=f"aon{h}") for h in range(NHC)]
            vhf = [vhpool.tile([128, S], F16, name=f"vhf{h}") for h in range(NHC)]
            vsl = [vslpool.tile([128, VSL_KC * 128], F16, name=f"vsl{h}")
                   for h in range(NHC)]

            ecnt = [0]

            def evict(out_ap, ps):
                eng = (nc.scalar.copy, nc.vector.tensor_copy)[ecnt[0] % 2]
                ecnt[0] += 1
                eng(out_ap, ps)

            vprep_done = [[False] * KC, [False] * KC]

            def vprep(h, kc):
                # lazy per-kc v transpose (+ sel-premult for far-capable kc)
                if kc >= KC or vprep_done[h][kc]:
                    return
                vprep_done[h][kc] = True
                tp = pspool.tile([128, 128], F16, tag="sc", bufs=3, name="tp")
                nc.tensor.transpose(tp, vts[h][:, kc * 128:(kc + 1) * 128], ident)
                dst = vhf[h][:, kc * 128:(kc + 1) * 128]
                evict(dst, tp)
                if kc < VSL_KC:
                    nc.vector.tensor_scalar_mul(
                        vsl[h][:, kc * 128:(kc + 1) * 128], dst,
                        svec32[:, kc:kc + 1],
                    )

            # out-proj slot machine: once head 1's softmax chain for qc is
            # done, its 16 (qt, oc) slots become pending; op_step() emits a
            # couple at a time between attention iterations so the
            # eviction-gated slots never serialize the in-order PE queue.
            # Banks: rotate over the freed av tags.
            op_pending = []
            op_tags = []
            op_n = [0]

            def op_enq(qc):
                op_tags.append(f"av{qc}")
                for qt in range(qc * 4, qc * 4 + 4):
                    for oc in range(OCC):
                        op_pending.append((qt, oc))

            def op_step(n):
                for _ in range(n):
                    if not op_pending:
                        return
                    qt, oc = op_pending.pop(0)
                    ps = pspool.tile([128, 512], F32,
                                     tag=op_tags[op_n[0] % len(op_tags)],
                                     name="wops")
                    op_n[0] += 1
                    for h in range(NHC):
                        nc.tensor.matmul(
                            ps, aon[h][:, qt * 128:(qt + 1) * 128],
                            wsb[h][:, oc * 512:(oc + 1) * 512],
                            start=(h == 0), stop=(h == NHC - 1),
                        )
                    ot = evpool.tile([128, 512], F16, tag="ot", name="ot")
                    evict(ot, ps)
                    nc.sync.dma_start(
                        out=part[qt * 128:(qt + 1) * 128,
                                 oc * 512:(oc + 1) * 512],
                        in_=ot,
                    )

            from collections import deque
            pend = deque()
            for h in range(NHC):
                vprep(h, 0)
                vprep(h, 1)
                avp = [
                    pspool.tile([128, 512], F32, tag=f"av{qc}", bufs=1,
                                name=f"av{qc}")
                    for qc in range(NQ)
                ]
                den128 = pspool.tile([128, 512], F32, tag="den", bufs=1,
                                     name="den128")

                def chain(qc, h=h, avp=avp, den128=den128):
                    # normalize qc: den broadcast via PE (one short ACT hop),
                    # reciprocal + multiply on DVE.
                    q0 = qc * 512
                    dq = drpool.tile([1, 512], F32R, tag=f"dq{qc}",
                                     name=f"dq{qc}")
                    nc.scalar.copy(dq, den128[32 * qc:32 * qc + 1, :])
                    rb = pspool.tile([128, 512], F32, tag="sc", bufs=3,
                                     name="rb")
                    nc.tensor.matmul(rb, ones1, dq, start=True, stop=True)
                    rbs = drpool.tile([128, 512], F32, tag="rbs", name="rbs")
                    rs = drpool.tile([128, 512], F32, tag="rs", name="rs")
                    nc.vector.reciprocal_approx_accurate(rbs, rb, rs)
                    nc.vector.scalar_tensor_tensor(
                        aon[h][:, q0:q0 + 512], rbs, 1.0, avp[qc],
                        op0=OP.mult, op1=OP.mult,
                    )
                    if h == NHC - 1:
                        op_enq(qc)

                def av_den(kc, qcs, far, ets, h=h, avp=avp, den128=den128,
                           chain=chain):
                    for qc in qcs:
                        lhs_av = (vsl[h][:, kc * 128:(kc + 1) * 128]
                                  if far[qc] else
                                  vhf[h][:, kc * 128:(kc + 1) * 128])
                        nc.tensor.matmul(
                            avp[qc], lhs_av, ets[qc],
                            start=(kc == 0), stop=(kc == (qc * 512 + 511) // 128),
                        )
                    for qc in qcs:
                        stop_kc = (qc * 512 + 511) // 128
                        lhs_den = svec[:, kc:kc + 1] if far[qc] else ones
                        nc.tensor.matmul(
                            den128[32 * qc:32 * qc + 1, :], lhs_den, ets[qc],
                            start=(kc == 0), stop=(kc == stop_kc),
                            tile_position=(0, 32 * qc),
                        )
                        if kc == stop_kc:
                            chain(qc)

                # software-pipelined by two kc stages; the deque spans
                # the head boundary so head 1's independent qk/exp work
                # fills head 0's thin-tail PE bubbles
                for kc in range(KC):
                    vprep(h, kc + 2)
                    k0 = kc * 128
                    qcs = [qc for qc in range(NQ) if k0 <= qc * 512 + 511]
                    far = {qc: qc * 512 > k0 + 127 + window for qc in qcs}
                    ets = {}
                    for qc in qcs:
                        q0 = qc * 512
                        q1 = q0 + 511
                        sps = pspool.tile([128, 512], F32, tag="sc", bufs=3,
                                          name="sps")
                        nc.tensor.matmul(
                            sps, ksb[h][:, kc * 128:(kc + 1) * 128],
                            qsb[h][:, q0:q0 + 512], start=True, stop=True,
                        )
                        et = etpool.tile([128, 512], F16, tag=f"et{qc}",
                                         name=f"et{qc}")
                        ets[qc] = et
                        nc.scalar.activation(et, sps, AF.Exp, scale=scale)
                        if far[qc]:
                            continue  # sel-mask folded into vsl/svec operands
                        if q0 < k0 + 128:
                            # causal: zero where q < k (iota - k < 0)
                            nc.gpsimd.affine_select(
                                out=et, in_=et, compare_op=OP.is_ge, fill=0.0,
                                base=q0 - k0, channel_multiplier=-1,
                                pattern=[[1, 512]],
                            )
                        if q1 > k0 + window:
                            nc.vector.scalar_tensor_tensor(
                                et, iota[:, q0:q0 + 512], hvec[:, kc:kc + 1], et,
                                op0=OP.is_le, op1=OP.mult,
                            )
                    pend.append((av_den, (kc, qcs, far, ets)))
                    if len(pend) > 2:
                        fn, args = pend.popleft()
                        fn(*args)
                        op_step(2)
            while pend:
                fn, args = pend.popleft()
                fn(*args)
                op_step(2)
            while op_pending:
                op_step(4)
    nc.compile()
    return nc


_CACHE = {}


def _get(name, builder, *args):
    key = (name,) + args
    if key not in _CACHE:
        _CACHE[key] = builder(*args)
    return _CACHE[key]


def _run(nc, in_maps):
    res = run_bass_kernel_spmd(
        nc, in_maps, core_ids=list(range(N_CORES)), trace=_TRACE["on"]
    )
    if _TRACE["on"] and res.exec_time_ns is not None:
        _TRACE["exec_ns"].append(res.exec_time_ns)
    return res.results


def kernel(hidden_states, Wq, Wk, Wv, Wo, Wq_ind, Wk_ind, head_weights,
           temperature_param):
    hidden_states = np.asarray(hidden_states, dtype=FP32)
    Wq, Wk, Wv, Wo = (np.asarray(a, dtype=FP32) for a in (Wq, Wk, Wv, Wo))
    Wq_ind = np.asarray(Wq_ind, dtype=FP32)
    Wk_ind = np.asarray(Wk_ind, dtype=FP32)
    head_weights = np.asarray(head_weights, dtype=FP32)

    B, S, H = hidden_states.shape
    assert B == 1 and H == HIDDEN
    CS = H // N_CORES
    D = IND_DIM
    HT = H // 128
    Wfq = Wq @ Wq_ind  # fused indexer weights (f32 host fuse)
    Wfk = Wk @ Wk_ind

    def pmajor(x):
        # (H, C) -> (128, HT*C): out[p, t*C+c] = x[t*128+p, c]
        C = x.shape[1]
        return np.ascontiguousarray(
            x.reshape(HT, 128, C).transpose(1, 0, 2).reshape(128, HT * C))

    hidT = pmajor(np.ascontiguousarray(hidden_states[0].T))

    # ---- Launch A: projections + indexer, head-parallel ----
    ncA = _get("A", build_fused, S, H, CS, D)
    inA = [
        {
            "hidT": hidT,
            "wq": pmajor(Wq[:, c * CS:(c + 1) * CS]),
            "wk": pmajor(Wk[:, c * CS:(c + 1) * CS]),
            "wv": pmajor(Wv[:, c * CS:(c + 1) * CS]),
            "wfq": pmajor(Wfq[:, c * D:(c + 1) * D]),
            "wfk": pmajor(Wfk[:, c * D:(c + 1) * D]),
        }
        for c in range(N_CORES)
    ]
    rA = _run(ncA, inA)
    rel = np.zeros(S, dtype=np.float64)
    for c in range(N_CORES):
        # rel arrives as relmat [128, QT]: rel[t*128+p] = relmat[p, t]
        rel += float(head_weights[c]) * \
            rA[c]["rel"].astype(np.float64).T.ravel()
    # exp(-temp) scaling is monotone; irrelevant for top-k selection.

    k_sel = min(MAX_SELECTED, S)
    top_idx = np.argpartition(-rel, k_sel - 1)[:k_sel]
    selected = np.zeros(S, dtype=bool)
    selected[top_idx] = True

    # ---- Launch B: masked attention + output projection, head-parallel ----
    BIG = float(2 * S + 1024)
    hi = np.where(selected, BIG, np.arange(S, dtype=np.float64) + LOCAL_WINDOW)
    hi = hi.astype(np.float16)
    selv = selected.astype(np.float16)
    iotar = np.broadcast_to(
        np.arange(S, dtype=np.float16)[None, :], (128, S)).copy()
    NHC = NUM_HEADS // N_CORES
    RW = NHC * HEAD_DIM
    ncB = _get("B", build_attn, S, H, NHC, HEAD_DIM, LOCAL_WINDOW)
    inB = [
        {
            "qTh": rA[c]["qT"],
            "kTh": rA[c]["kT"],
            "vTh": rA[c]["vT"],
            "woh": np.ascontiguousarray(Wo[c * RW:(c + 1) * RW]).astype(
                np.float16),
            "iotar": iotar,
            "hivec": hi,
            "selv": selv,
            "onesrow": np.ones(128, dtype=np.float32),
        }
        for c in range(N_CORES)
    ]
    rB = _run(ncB, inB)
    out = rB[0]["part"].astype(np.float32)
    for c in range(1, N_CORES):
        out += rB[c]["part"].astype(np.float32)
    return out.reshape(B, S, H)


# revision 47
# speedup vs baseline: 1.0332x; 1.0332x over previous
"""DeepSeek sparse attention on 8 Trainium2 NeuronCores (Bass/Tile).

Strategy (2 SPMD launches, head-parallel, f32r indexer + fp16 attention):

  host: fuse indexer weights through the attention projections:
      Wfq = Wq @ Wq_ind, Wfk = Wk @ Wk_ind  (so the indexer reads
      hidden directly and needs no q_lin/k_lin round trip).
  A   (fused projections + indexer): core c keeps hidden^T resident in
      SBUF and runs: wfq -> qp_c^T, wfk -> kp_c^T (indexer head c,
      f32r), then the relu(qp.kp) score reduction -> rel_c interleaved
      with the wq pass (the PE fills score-drain stalls with wq
      matmuls), then wk/wv passes.  qT/kT/vT (fp16) are the core's own
      2 attention heads.
  host: rel = sum_c w_c * rel_c; top-1024 keys -> selected mask ->
      hi[k] threshold vector (fp16).
  B   (attention, fp16): core c computes softmax attention for its 2
      heads, masked causal/local/selected; the output projection for
      query chunk qc is interleaved right after head 1's softmax chain
      for qc completes, so out-DMA overlaps compute.
  host: out = sum_c partial_c.

The indexer path stays f32r end-to-end: the top-k boundary gaps are
~1e-5 relative, and selection flips cost ~1e-2 output error each.
The attention path is fp16 (storage) with f32 PE accumulation.
"""

import math

import numpy as np

import concourse.bass as bass
import concourse.mybir as mybir
from concourse import bacc
from concourse.tile import TileContext
from concourse.masks import make_identity
from concourse.bass_utils import run_bass_kernel_spmd

# Problem constants (hardcoded per contract)
HIDDEN = 2048
NUM_HEADS = 16
HEAD_DIM = 128
NUM_IND_HEADS = 8
IND_DIM = HIDDEN // NUM_IND_HEADS  # 256
MAX_SELECTED = 1024
LOCAL_WINDOW = 512
N_CORES = 8

F32 = mybir.dt.float32
F32R = mybir.dt.float32r
F16 = mybir.dt.float16
F8 = mybir.dt.float8e4
FP32 = np.float32

import ml_dtypes

E4M3 = ml_dtypes.float8_e4m3fn
SH = 8.0    # fp8 hidden scale
SW = 512.0  # fp8 weight scale (descale 1/(SH*SW) folded into evictions)

_TRACE = {"on": False, "exec_ns": []}


def build_fused(S=2048, H=HIDDEN, CS=HIDDEN // N_CORES, D=IND_DIM):
    """Launch A: hidden^T resident; qp/kp passes, scores x wq interleave,
    then wk/wv passes."""
    nc = bacc.Bacc("TRN2", target_bir_lowering=False, debug=False)
    HT, NQ, QT, DC, MC = H // 128, S // 512, S // 128, D // 128, CS // 128
    # all inputs arrive partition-major (host pre-rearranged): x[p, t*C+c]
    # = orig[t*128+p, c] — plain 2D DMAs, 128 descriptors each.
    hidT = nc.dram_tensor("hidT", [128, HT * S], F32R, kind="ExternalInput")
    wq = nc.dram_tensor("wq", [128, HT * CS], F32R, kind="ExternalInput")
    wk = nc.dram_tensor("wk", [128, HT * CS], F32R, kind="ExternalInput")
    wv = nc.dram_tensor("wv", [128, HT * CS], F32R, kind="ExternalInput")
    wfq = nc.dram_tensor("wfq", [128, HT * D], F32R, kind="ExternalInput")
    wfk = nc.dram_tensor("wfk", [128, HT * D], F32R, kind="ExternalInput")
    qT = nc.dram_tensor("qT", [CS, S], F16, kind="ExternalOutput")
    kT = nc.dram_tensor("kT", [CS, S], F16, kind="ExternalOutput")
    vT = nc.dram_tensor("vT", [CS, S], F16, kind="ExternalOutput")
    # rel written as the raw [128, QT] relmat; host unshuffles (p,t)->t*128+p
    rel = nc.dram_tensor("rel", [128, QT], F32, kind="ExternalOutput")

    AF = mybir.ActivationFunctionType

    with TileContext(nc) as tc:
        with (
            tc.tile_pool(name="hid", bufs=1) as hpool,
            tc.tile_pool(name="wres", bufs=1) as wrpool,
            tc.tile_pool(name="wt", bufs=2) as wpool,
            tc.tile_pool(name="proj", bufs=1) as ppool,
            tc.tile_pool(name="ev", bufs=4) as opool,
            tc.tile_pool(name="scr", bufs=2) as scpool,
            tc.tile_pool(name="rc", bufs=2) as rcpool,
            tc.tile_pool(name="rm", bufs=1) as rmpool,
            tc.tile_pool(name="ps", bufs=1, space="PSUM") as pspool,
        ):
            # hidden^T resident, loaded strip by strip (1 MB each) so the
            # first matmuls only wait on strip 0.
            hidc = [None] * HT

            def load_hid(t):
                hc = hpool.tile([128, S], F32R, name=f"hidc{t}")
                nc.sync.dma_start(out=hc, in_=hidT[:, t * S:(t + 1) * S])
                hidc[t] = hc

            WG = 4  # weight strips per streamed DMA group

            def load_wgroup(wd, g):
                w = wpool.tile([128, WG * CS], F32R, tag="ws", name="ws")
                nc.sync.dma_start(
                    out=w, in_=wd[:, g * WG * CS:(g + 1) * WG * CS])
                return w

            # DMA issue order = consumption order: hid0, wfq g0, hid1-2,
            # wfq g1, hid3-5, g2, hid6-8, g3, hid9-15.  wfk prefetched
            # during the qp pass; wq (full 2 MB resident) during kp.
            load_hid(0)
            wfq_groups = [load_wgroup(wfq, 0), None, None, None]
            load_hid(1), load_hid(2)
            wfq_groups[1] = load_wgroup(wfq, 1)
            load_hid(3), load_hid(4), load_hid(5)
            wfq_groups[2] = load_wgroup(wfq, 2)
            load_hid(6), load_hid(7), load_hid(8)
            wfq_groups[3] = load_wgroup(wfq, 3)
            for t in range(9, HT):
                load_hid(t)

            def hstrip(t):
                return hidc[t]

            qpt = [ppool.tile([128, S], F32R, name=f"qpt{i}") for i in range(DC)]
            kpt = [ppool.tile([128, S], F32R, name=f"kpt{i}") for i in range(DC)]

            PTAGS = [f"m{i}" for i in range(4)] + [f"s{i}" for i in range(4)]

            def proj_pass(wgroups, wd, sink, prefetch=None):
                # full 8-bank pass over all (mc, qc); streamed weight groups
                psq = [
                    pspool.tile([128, 512], F32, tag=PTAGS[i], name=PTAGS[i])
                    for i in range(MC * NQ)
                ]
                for t in range(HT):
                    g, tl = t // WG, t % WG
                    if wgroups[g] is None:
                        wgroups[g] = load_wgroup(wd, g)
                    if tl == 0 and g + 1 < len(wgroups) and wgroups[g + 1] is None:
                        wgroups[g + 1] = load_wgroup(wd, g + 1)
                    if prefetch is not None and t == HT // 2:
                        prefetch()
                    w = wgroups[g][:, tl * CS:(tl + 1) * CS]
                    rhs = hstrip(t)
                    for mc in range(MC):
                        for qc in range(NQ):
                            nc.tensor.matmul(
                                psq[mc * NQ + qc],
                                w[:, mc * 128:(mc + 1) * 128],
                                rhs[:, qc * 512:(qc + 1) * 512],
                                start=(t == 0), stop=(t == HT - 1),
                            )
                for mc in range(MC):
                    for qc in range(NQ):
                        sink(mc, qc, psq[mc * NQ + qc])

            ecnt = [0]

            def copy3(out_ap, ps):
                # psum->sbuf copies alternate ACT/DVE (gpsimd cannot read
                # PSUM — neuronxcc rejects it)
                eng = (nc.scalar.copy, nc.vector.tensor_copy)[ecnt[0] % 2]
                ecnt[0] += 1
                eng(out_ap, ps)

            def sink_proj(dst):
                def s(mc, qc, ps):
                    copy3(dst[mc][:, qc * 512:(qc + 1) * 512], ps)
                return s

            def sink_out(odram):
                def s(mc, qc, ps):
                    ot = opool.tile([128, 512], F16, tag="ot", name="ot")
                    copy3(ot, ps)
                    nc.gpsimd.dma_start(
                        out=odram[mc * 128:(mc + 1) * 128,
                                  qc * 512:(qc + 1) * 512],
                        in_=ot,
                    )
                return s

            # --- qp pass; prefetch wfk groups 0/1 halfway through ---
            wfk_groups = [None, None, None, None]

            def pf_wfk():
                wfk_groups[0] = load_wgroup(wfk, 0)
                wfk_groups[1] = load_wgroup(wfk, 1)

            proj_pass(wfq_groups, wfq, sink_proj(qpt), prefetch=pf_wfk)

            # --- kp pass; prefetch resident wq (2 MB) halfway through ---
            wq_res = [None]

            def pf_wq():
                wr = wrpool.tile([128, HT * CS], F32R, tag="wres", name="wqr")
                nc.sync.dma_start(out=wr, in_=wq[:, :])
                wq_res[0] = wr

            proj_pass(wfk_groups, wfk, sink_proj(kpt), prefetch=pf_wq)

            # --- scores x wq interleave ---
            # scores: per qt, 8 matmuls into 4 banks (s0..s3); the relu
            # reduction drains on ACT (tiles 0/1) + DVE (tiles 2/3).  The
            # drain rate (~2.5us/qt) exceeds the PE's 1.8us/qt, so wq
            # matmul chains (single-bank, 16-deep accumulation per
            # (mc,qc)) are interleaved to keep the PE busy.
            relmat = rmpool.tile([128, QT], F32, name="relmat")

            def score_qt(qt):
                spss = [
                    pspool.tile([128, 512], F32, tag=f"s{kc}", name="sps")
                    for kc in range(NQ)
                ]
                for dcc in range(DC):
                    for kc in range(NQ):
                        nc.tensor.matmul(
                            spss[kc],
                            qpt[dcc][:, qt * 128:(qt + 1) * 128],
                            kpt[dcc][:, kc * 512:(kc + 1) * 512],
                            start=(dcc == 0), stop=(dcc == DC - 1),
                        )
                relcols = rcpool.tile([128, NQ + 1], F32, tag="relcols",
                                      name="relcols")
                for kc in (0, 1):
                    scratch = scpool.tile([128, 512], F16, tag=f"scr{kc}",
                                          name="scratch")
                    nc.scalar.activation(
                        scratch, spss[kc], AF.Relu,
                        accum_out=relcols[:, kc:kc + 1],
                    )
                wide = scpool.tile([128, 1024], F16, tag="scrw", name="scrw")
                nc.vector.tensor_scalar_max(wide[:, 0:512], spss[2], 0.0)
                nc.vector.tensor_scalar_max(wide[:, 512:1024], spss[3], 0.0)
                nc.vector.tensor_reduce(
                    relcols[:, 2:3], wide,
                    axis=mybir.AxisListType.X, op=mybir.AluOpType.add,
                )
                relscr = rcpool.tile([128, 3], F32, tag="relscr",
                                     name="relscr")
                nc.scalar.activation(
                    relscr, relcols[:, 0:3], AF.Copy,
                    accum_out=relmat[:, qt:qt + 1],
                )

            # wq interleave: 8 chains of (mc,qc), each 16 accumulating
            # matmuls into one rotating bank (m0..m3).
            wq_sink = sink_out(qT)
            wq_chains = [(mc, qc) for mc in range(MC) for qc in range(NQ)]
            wq_state = {"chain": 0, "t": 0, "ps": None}

            def wq_step(nmm):
                # emit nmm wq matmuls (advancing chain state)
                wr = wq_res[0]
                for _ in range(nmm):
                    ci = wq_state["chain"]
                    if ci >= len(wq_chains):
                        return
                    mc, qc = wq_chains[ci]
                    t = wq_state["t"]
                    if t == 0:
                        wq_state["ps"] = pspool.tile(
                            [128, 512], F32, tag=f"m{ci % 4}", name="wqps")
                    nc.tensor.matmul(
                        wq_state["ps"],
                        wr[:, t * CS + mc * 128:t * CS + mc * 128 + 128],
                        hstrip(t)[:, qc * 512:(qc + 1) * 512],
                        start=(t == 0), stop=(t == HT - 1),
                    )
                    if t == HT - 1:
                        wq_sink(mc, qc, wq_state["ps"])
                        wq_state["chain"] = ci + 1
                        wq_state["t"] = 0
                    else:
                        wq_state["t"] = t + 1

            wk_groups = [None, None, None, None]

            def pf_wk():
                wk_groups[0] = load_wgroup(wk, 0)
                wk_groups[1] = load_wgroup(wk, 1)

            for qt in range(QT):
                score_qt(qt)
                if qt == QT - 4:
                    pf_wk()
                wq_step(7)  # 16 qt x 7 ~= 128 posted; rest drained after
            wq_step(128)
            nc.sync.dma_start(out=rel[:, :], in_=relmat)

            # --- wk / wv full 8-bank passes ---
            wv_groups = [None, None, None, None]

            def pf_wv():
                wv_groups[0] = load_wgroup(wv, 0)
                wv_groups[1] = load_wgroup(wv, 1)

            proj_pass(wk_groups, wk, sink_out(kT), prefetch=pf_wv)
            proj_pass(wv_groups, wv, sink_out(vT))
    nc.compile()
    return nc


def build_attn(S=2048, H=HIDDEN, NHC=NUM_HEADS // N_CORES, HD=HEAD_DIM,
               window=LOCAL_WINDOW):
    """Launch B: per-core (2 heads) masked softmax attention + out-proj,
    with the out-proj interleaved into head 1's attention."""
    nc = bacc.Bacc("TRN2", target_bir_lowering=False, debug=False)
    KC, NQ, QT, OCC = S // 128, S // 512, S // 128, H // 512
    qTh = nc.dram_tensor("qTh", [NHC * HD, S], F16, kind="ExternalInput")
    kTh = nc.dram_tensor("kTh", [NHC * HD, S], F16, kind="ExternalInput")
    vTh = nc.dram_tensor("vTh", [NHC * HD, S], F16, kind="ExternalInput")
    woh = nc.dram_tensor("woh", [NHC * HD, H], F16, kind="ExternalInput")
    iotar = nc.dram_tensor("iotar", [128, S], F16, kind="ExternalInput")
    hivec = nc.dram_tensor("hivec", [S], F16, kind="ExternalInput")
    selv = nc.dram_tensor("selv", [S], F16, kind="ExternalInput")
    onesrow = nc.dram_tensor("onesrow", [128], F32R, kind="ExternalInput")
    part = nc.dram_tensor("part", [S, H], F16, kind="ExternalOutput")

    scale = 1.0 / math.sqrt(HD)
    AF = mybir.ActivationFunctionType
    OP = mybir.AluOpType
    VSL_KC = 8  # far blocks only exist for kc <= 7

    with TileContext(nc) as tc:
        with (
            tc.tile_pool(name="const", bufs=1) as cpool,
            tc.tile_pool(name="qk", bufs=1) as qkpool,
            tc.tile_pool(name="vt", bufs=1) as vtpool,
            tc.tile_pool(name="vh", bufs=1) as vhpool,
            tc.tile_pool(name="vsl", bufs=1) as vslpool,
            tc.tile_pool(name="et", bufs=3) as etpool,
            tc.tile_pool(name="aon", bufs=1) as aopool,
            tc.tile_pool(name="dr", bufs=2) as drpool,
            tc.tile_pool(name="ev", bufs=4) as evpool,
            tc.tile_pool(name="ps", bufs=1, space="PSUM") as pspool,
        ):
            # DMA priority: head-0 q/k first (gates first matmul), then
            # v0, consts, head-1 tensors, wo.
            qsb, ksb, vts = [None, None], [None, None], [None, None]
            for h in range(NHC):
                qsb[h] = qkpool.tile([128, S], F16, name=f"qsb{h}")
                nc.sync.dma_start(out=qsb[h], in_=qTh[h * HD:(h + 1) * HD, :])
                ksb[h] = qkpool.tile([128, S], F16, name=f"ksb{h}")
                nc.sync.dma_start(out=ksb[h], in_=kTh[h * HD:(h + 1) * HD, :])
                if h == 0:
                    vts[0] = vtpool.tile([128, S], F16, name="vts0")
                    nc.sync.dma_start(out=vts[0], in_=vTh[0:HD, :])
                    ident = cpool.tile([128, 128], F16, name="ident")
                    make_identity(nc, ident)
                    iota = cpool.tile([128, S], F16, name="iota")
                    nc.sync.dma_start(out=iota, in_=iotar[:, :])
                    ones = cpool.tile([128, 1], F16, name="ones")
                    nc.vector.memset(ones, 1.0)
                    hvec = cpool.tile([128, KC], F16, name="hvec")
                    nc.sync.dma_start(
                        out=hvec, in_=hivec.rearrange("(t p) -> p t", p=128))
                    svec = cpool.tile([128, KC], F16, name="svec")
                    nc.sync.dma_start(
                        out=svec, in_=selv.rearrange("(t p) -> p t", p=128))
                    svec32 = cpool.tile([128, KC], F32, name="svec32")
                    nc.vector.tensor_copy(svec32, svec)
                    ones1 = cpool.tile([1, 128], F32R, name="ones1")
                    nc.sync.dma_start(out=ones1, in_=onesrow[None, :])
            vts[1] = vtpool.tile([128, S], F16, name="vts1")
            nc.sync.dma_start(out=vts[1], in_=vTh[HD:2 * HD, :])
            wsb = []
            for h in range(NHC):
                w = qkpool.tile([128, H], F16, name=f"wsb{h}")
                nc.sync.dma_start(out=w, in_=woh[h * HD:(h + 1) * HD, :])
                wsb.append(w)

            aon = [aopool.tile([128, S], F16, name=f"aon{h}") for h in range(NHC)]
            vhf = [vhpool.tile([128, S], F16, name=f"vhf{h}") for h in range(NHC)]
            vsl = [vslpool.tile([128, VSL_KC * 128], F16, name=f"vsl{h}")
                   for h in range(NHC)]

            ecnt = [0]

            def evict(out_ap, ps):
                eng = (nc.scalar.copy, nc.vector.tensor_copy)[ecnt[0] % 2]
                ecnt[0] += 1
                eng(out_ap, ps)

            vprep_done = [[False] * KC, [False] * KC]

            def vprep(h, kc):
                # lazy per-kc v transpose (+ sel-premult for far-capable kc)
                if kc >= KC or vprep_done[h][kc]:
                    return
                vprep_done[h][kc] = True
                tp = pspool.tile([128, 128], F16, tag="sc", bufs=3, name="tp")
                nc.tensor.transpose(tp, vts[h][:, kc * 128:(kc + 1) * 128], ident)
                dst = vhf[h][:, kc * 128:(kc + 1) * 128]
                evict(dst, tp)
                if kc < VSL_KC:
                    nc.vector.tensor_scalar_mul(
                        vsl[h][:, kc * 128:(kc + 1) * 128], dst,
                        svec32[:, kc:kc + 1],
                    )

            # out-proj slot machine: once head 1's softmax chain for qc is
            # done, its 16 (qt, oc) slots become pending; op_step() emits a
            # couple at a time between attention iterations so the
            # eviction-gated slots never serialize the in-order PE queue.
            # Banks: rotate over the freed av tags.
            op_pending = []
            op_tags = []
            op_n = [0]

            def op_enq(qc):
                op_tags.append(f"av{qc}")
                for qt in range(qc * 4, qc * 4 + 4):
                    for oc in range(OCC):
                        op_pending.append((qt, oc))

            def op_step(n):
                for _ in range(n):
                    if not op_pending:
                        return
                    qt, oc = op_pending.pop(0)
                    ps = pspool.tile([128, 512], F32,
                                     tag=op_tags[op_n[0] % len(op_tags)],
                                     name="wops")
                    op_n[0] += 1
                    for h in range(NHC):
                        nc.tensor.matmul(
                            ps, aon[h][:, qt * 128:(qt + 1) * 128],
                            wsb[h][:, oc * 512:(oc + 1) * 512],
                            start=(h == 0), stop=(h == NHC - 1),
                        )
                    ot = evpool.tile([128, 512], F16, tag="ot", name="ot")
                    evict(ot, ps)
                    nc.sync.dma_start(
                        out=part[qt * 128:(qt + 1) * 128,
                                 oc * 512:(oc + 1) * 512],
                        in_=ot,
                    )

            from collections import deque
            pend = deque()
            for h in range(NHC):
                vprep(h, 0)
                vprep(h, 1)
                avp = [
                    pspool.tile([128, 512], F32, tag=f"av{qc}", bufs=1,
                                name=f"av{qc}")
                    for qc in range(NQ)
                ]
                den128 = pspool.tile([128, 512], F32, tag="den", bufs=1,
                                     name="den128")

                def chain(qc, h=h, avp=avp, den128=den128):
                    # normalize qc: den broadcast via PE (one short ACT hop),
                    # reciprocal + multiply on DVE.
                    q0 = qc * 512
                    dq = drpool.tile([1, 512], F32R, tag=f"dq{qc}",
                                     name=f"dq{qc}")
                    nc.scalar.copy(dq, den128[32 * qc:32 * qc + 1, :])
                    rb = pspool.tile([128, 512], F32, tag="sc", bufs=3,
                                     name="rb")
                    nc.tensor.matmul(rb, ones1, dq, start=True, stop=True)
                    rbs = drpool.tile([128, 512], F32, tag="rbs", name="rbs")
                    rs = drpool.tile([128, 512], F32, tag="rs", name="rs")
                    nc.vector.reciprocal_approx_accurate(rbs, rb, rs)
                    nc.vector.scalar_tensor_tensor(
                        aon[h][:, q0:q0 + 512], rbs, 1.0, avp[qc],
                        op0=OP.mult, op1=OP.mult,
                    )
                    if h == NHC - 1:
                        op_enq(qc)

                def av_den(kc, qcs, far, ets, h=h, avp=avp, den128=den128,
                           chain=chain):
                    for qc in qcs:
                        lhs_av = (vsl[h][:, kc * 128:(kc + 1) * 128]
                                  if far[qc] else
                                  vhf[h][:, kc * 128:(kc + 1) * 128])
                        nc.tensor.matmul(
                            avp[qc], lhs_av, ets[qc],
                            start=(kc == 0), stop=(kc == (qc * 512 + 511) // 128),
                        )
                    for qc in qcs:
                        stop_kc = (qc * 512 + 511) // 128
                        lhs_den = svec[:, kc:kc + 1] if far[qc] else ones
                        nc.tensor.matmul(
                            den128[32 * qc:32 * qc + 1, :], lhs_den, ets[qc],
                            start=(kc == 0), stop=(kc == stop_kc),
                            tile_position=(0, 32 * qc),
                        )
                        if kc == stop_kc:
                            chain(qc)

                # software-pipelined by two kc stages; the deque spans
                # the head boundary so head 1's independent qk/exp work
                # fills head 0's thin-tail PE bubbles
                for kc in range(KC):
                    vprep(h, kc + 2)
                    k0 = kc * 128
                    qcs = [qc for qc in range(NQ) if k0 <= qc * 512 + 511]
                    far = {qc: qc * 512 > k0 + 127 + window for qc in qcs}
                    ets = {}
                    for qc in qcs:
                        q0 = qc * 512
                        q1 = q0 + 511
                        sps = pspool.tile([128, 512], F32, tag="sc", bufs=3,
                                          name="sps")
                        nc.tensor.matmul(
                            sps, ksb[h][:, kc * 128:(kc + 1) * 128],
                            qsb[h][:, q0:q0 + 512], start=True, stop=True,
                        )
                        et = etpool.tile([128, 512], F16, tag=f"et{qc}",
                                         bufs=4, name=f"et{qc}")
                        ets[qc] = et
                        nc.scalar.activation(et, sps, AF.Exp, scale=scale)
                        if far[qc]:
                            continue  # sel-mask folded into vsl/svec operands
                        if q0 < k0 + 128:
                            # causal: zero where q < k (iota - k < 0)
                            nc.gpsimd.affine_select(
                                out=et, in_=et, compare_op=OP.is_ge, fill=0.0,
                                base=q0 - k0, channel_multiplier=-1,
                                pattern=[[1, 512]],
                            )
                        if q1 > k0 + window:
                            nc.vector.scalar_tensor_tensor(
                                et, iota[:, q0:q0 + 512], hvec[:, kc:kc + 1], et,
                                op0=OP.is_le, op1=OP.mult,
                            )
                    pend.append((av_den, (kc, qcs, far, ets)))
                    # h0's last iterations are stall-prone (thin kc tail);
                    # defer their av/den into h1's work-rich start, where
                    # their ACT/DVE chains have already completed
                    maxd = 5 if (h == 0 and kc >= KC - 3) else 2
                    while len(pend) > maxd:
                        fn, args = pend.popleft()
                        fn(*args)
                        op_step(2)
            while pend:
                fn, args = pend.popleft()
                fn(*args)
                op_step(2)
            while op_pending:
                op_step(4)
    nc.compile()
    return nc


_CACHE = {}


def _get(name, builder, *args):
    key = (name,) + args
    if key not in _CACHE:
        _CACHE[key] = builder(*args)
    return _CACHE[key]


def _run(nc, in_maps):
    res = run_bass_kernel_spmd(
        nc, in_maps, core_ids=list(range(N_CORES)), trace=_TRACE["on"]
    )
    if _TRACE["on"] and res.exec_time_ns is not None:
        _TRACE["exec_ns"].append(res.exec_time_ns)
    return res.results


def kernel(hidden_states, Wq, Wk, Wv, Wo, Wq_ind, Wk_ind, head_weights,
           temperature_param):
    hidden_states = np.asarray(hidden_states, dtype=FP32)
    Wq, Wk, Wv, Wo = (np.asarray(a, dtype=FP32) for a in (Wq, Wk, Wv, Wo))
    Wq_ind = np.asarray(Wq_ind, dtype=FP32)
    Wk_ind = np.asarray(Wk_ind, dtype=FP32)
    head_weights = np.asarray(head_weights, dtype=FP32)

    B, S, H = hidden_states.shape
    assert B == 1 and H == HIDDEN
    CS = H // N_CORES
    D = IND_DIM
    HT = H // 128
    Wfq = Wq @ Wq_ind  # fused indexer weights (f32 host fuse)
    Wfk = Wk @ Wk_ind

    def pmajor(x):
        # (H, C) -> (128, HT*C): out[p, t*C+c] = x[t*128+p, c]
        C = x.shape[1]
        return np.ascontiguousarray(
            x.reshape(HT, 128, C).transpose(1, 0, 2).reshape(128, HT * C))

    hidT = pmajor(np.ascontiguousarray(hidden_states[0].T))

    # ---- Launch A: projections + indexer, head-parallel ----
    ncA = _get("A", build_fused, S, H, CS, D)
    inA = [
        {
            "hidT": hidT,
            "wq": pmajor(Wq[:, c * CS:(c + 1) * CS]),
            "wk": pmajor(Wk[:, c * CS:(c + 1) * CS]),
            "wv": pmajor(Wv[:, c * CS:(c + 1) * CS]),
            "wfq": pmajor(Wfq[:, c * D:(c + 1) * D]),
            "wfk": pmajor(Wfk[:, c * D:(c + 1) * D]),
        }
        for c in range(N_CORES)
    ]
    rA = _run(ncA, inA)
    rel = np.zeros(S, dtype=np.float64)
    for c in range(N_CORES):
        # rel arrives as relmat [128, QT]: rel[t*128+p] = relmat[p, t]
        rel += float(head_weights[c]) * \
            rA[c]["rel"].astype(np.float64).T.ravel()
    # exp(-temp) scaling is monotone; irrelevant for top-k selection.

    k_sel = min(MAX_SELECTED, S)
    top_idx = np.argpartition(-rel, k_sel - 1)[:k_sel]
    selected = np.zeros(S, dtype=bool)
    selected[top_idx] = True

    # ---- Launch B: masked attention + output projection, head-parallel ----
    BIG = float(2 * S + 1024)
    hi = np.where(selected, BIG, np.arange(S, dtype=np.float64) + LOCAL_WINDOW)
    hi = hi.astype(np.float16)
    selv = selected.astype(np.float16)
    iotar = np.broadcast_to(
        np.arange(S, dtype=np.float16)[None, :], (128, S)).copy()
    NHC = NUM_HEADS // N_CORES
    RW = NHC * HEAD_DIM
    ncB = _get("B", build_attn, S, H, NHC, HEAD_DIM, LOCAL_WINDOW)
    inB = [
        {
            "qTh": rA[c]["qT"],
            "kTh": rA[c]["kT"],
            "vTh": rA[c]["vT"],
            "woh": np.ascontiguousarray(Wo[c * RW:(c + 1) * RW]).astype(
                np.float16),
            "iotar": iotar,
            "hivec": hi,
            "selv": selv,
            "onesrow": np.ones(128, dtype=np.float32),
        }
        for c in range(N_CORES)
    ]
    rB = _run(ncB, inB)
    out = rB[0]["part"].astype(np.float32)
    for c in range(1, N_CORES):
        out += rB[c]["part"].astype(np.float32)
    return out.reshape(B, S, H)


# revision 48
# speedup vs baseline: 1.0358x; 1.0025x over previous
"""DeepSeek sparse attention on 8 Trainium2 NeuronCores (Bass/Tile).

Strategy (2 SPMD launches, head-parallel, f32r indexer + fp16 attention):

  host: fuse indexer weights through the attention projections:
      Wfq = Wq @ Wq_ind, Wfk = Wk @ Wk_ind  (so the indexer reads
      hidden directly and needs no q_lin/k_lin round trip).
  A   (fused projections + indexer): core c keeps hidden^T resident in
      SBUF and runs: wfq -> qp_c^T, wfk -> kp_c^T (indexer head c,
      f32r), then the relu(qp.kp) score reduction -> rel_c interleaved
      with the wq pass (the PE fills score-drain stalls with wq
      matmuls), then wk/wv passes.  qT/kT/vT (fp16) are the core's own
      2 attention heads.
  host: rel = sum_c w_c * rel_c; top-1024 keys -> selected mask ->
      hi[k] threshold vector (fp16).
  B   (attention, fp16): core c computes softmax attention for its 2
      heads, masked causal/local/selected; the output projection for
      query chunk qc is interleaved right after head 1's softmax chain
      for qc completes, so out-DMA overlaps compute.
  host: out = sum_c partial_c.

The indexer path stays f32r end-to-end: the top-k boundary gaps are
~1e-5 relative, and selection flips cost ~1e-2 output error each.
The attention path is fp16 (storage) with f32 PE accumulation.
"""

import math

import numpy as np

import concourse.bass as bass
import concourse.mybir as mybir
from concourse import bacc
from concourse.tile import TileContext
from concourse.masks import make_identity
from concourse.bass_utils import run_bass_kernel_spmd

# Problem constants (hardcoded per contract)
HIDDEN = 2048
NUM_HEADS = 16
HEAD_DIM = 128
NUM_IND_HEADS = 8
IND_DIM = HIDDEN // NUM_IND_HEADS  # 256
MAX_SELECTED = 1024
LOCAL_WINDOW = 512
N_CORES = 8

F32 = mybir.dt.float32
F32R = mybir.dt.float32r
F16 = mybir.dt.float16
F8 = mybir.dt.float8e4
FP32 = np.float32

import ml_dtypes

E4M3 = ml_dtypes.float8_e4m3fn
SH = 8.0    # fp8 hidden scale
SW = 512.0  # fp8 weight scale (descale 1/(SH*SW) folded into evictions)

_TRACE = {"on": False, "exec_ns": []}


def build_fused(S=2048, H=HIDDEN, CS=HIDDEN // N_CORES, D=IND_DIM):
    """Launch A: hidden^T resident; qp/kp passes, scores x wq interleave,
    then wk/wv passes."""
    nc = bacc.Bacc("TRN2", target_bir_lowering=False, debug=False)
    HT, NQ, QT, DC, MC = H // 128, S // 512, S // 128, D // 128, CS // 128
    # all inputs arrive partition-major (host pre-rearranged): x[p, t*C+c]
    # = orig[t*128+p, c] — plain 2D DMAs, 128 descriptors each.
    hidT = nc.dram_tensor("hidT", [128, HT * S], F32R, kind="ExternalInput")
    wq = nc.dram_tensor("wq", [128, HT * CS], F32R, kind="ExternalInput")
    wk = nc.dram_tensor("wk", [128, HT * CS], F32R, kind="ExternalInput")
    wv = nc.dram_tensor("wv", [128, HT * CS], F32R, kind="ExternalInput")
    wfq = nc.dram_tensor("wfq", [128, HT * D], F32R, kind="ExternalInput")
    wfk = nc.dram_tensor("wfk", [128, HT * D], F32R, kind="ExternalInput")
    qT = nc.dram_tensor("qT", [CS, S], F16, kind="ExternalOutput")
    kT = nc.dram_tensor("kT", [CS, S], F16, kind="ExternalOutput")
    vT = nc.dram_tensor("vT", [CS, S], F16, kind="ExternalOutput")
    # rel written as the raw [128, QT] relmat; host unshuffles (p,t)->t*128+p
    rel = nc.dram_tensor("rel", [128, QT], F32, kind="ExternalOutput")

    AF = mybir.ActivationFunctionType

    with TileContext(nc) as tc:
        with (
            tc.tile_pool(name="hid", bufs=1) as hpool,
            tc.tile_pool(name="wres", bufs=1) as wrpool,
            tc.tile_pool(name="wt", bufs=2) as wpool,
            tc.tile_pool(name="proj", bufs=1) as ppool,
            tc.tile_pool(name="ev", bufs=4) as opool,
            tc.tile_pool(name="scr", bufs=2) as scpool,
            tc.tile_pool(name="rc", bufs=2) as rcpool,
            tc.tile_pool(name="rm", bufs=1) as rmpool,
            tc.tile_pool(name="ps", bufs=1, space="PSUM") as pspool,
        ):
            # hidden^T resident, loaded strip by strip (1 MB each) so the
            # first matmuls only wait on strip 0.
            hidc = [None] * HT

            def load_hid(t):
                hc = hpool.tile([128, S], F32R, name=f"hidc{t}")
                nc.sync.dma_start(out=hc, in_=hidT[:, t * S:(t + 1) * S])
                hidc[t] = hc

            WG = 4  # weight strips per streamed DMA group

            def load_wgroup(wd, g):
                w = wpool.tile([128, WG * CS], F32R, tag="ws", name="ws")
                nc.sync.dma_start(
                    out=w, in_=wd[:, g * WG * CS:(g + 1) * WG * CS])
                return w

            # DMA issue order = consumption order: hid0, wfq g0, hid1-2,
            # wfq g1, hid3-5, g2, hid6-8, g3, hid9-15.  wfk prefetched
            # during the qp pass; wq (full 2 MB resident) during kp.
            load_hid(0)
            wfq_groups = [load_wgroup(wfq, 0), None, None, None]
            load_hid(1), load_hid(2)
            wfq_groups[1] = load_wgroup(wfq, 1)
            load_hid(3), load_hid(4), load_hid(5)
            wfq_groups[2] = load_wgroup(wfq, 2)
            load_hid(6), load_hid(7), load_hid(8)
            wfq_groups[3] = load_wgroup(wfq, 3)
            for t in range(9, HT):
                load_hid(t)

            def hstrip(t):
                return hidc[t]

            qpt = [ppool.tile([128, S], F32R, name=f"qpt{i}") for i in range(DC)]
            kpt = [ppool.tile([128, S], F32R, name=f"kpt{i}") for i in range(DC)]

            PTAGS = [f"m{i}" for i in range(4)] + [f"s{i}" for i in range(4)]

            def proj_pass(wgroups, wd, sink, prefetch=None):
                # full 8-bank pass over all (mc, qc); streamed weight groups
                psq = [
                    pspool.tile([128, 512], F32, tag=PTAGS[i], name=PTAGS[i])
                    for i in range(MC * NQ)
                ]
                for t in range(HT):
                    g, tl = t // WG, t % WG
                    if wgroups[g] is None:
                        wgroups[g] = load_wgroup(wd, g)
                    if tl == 0 and g + 1 < len(wgroups) and wgroups[g + 1] is None:
                        wgroups[g + 1] = load_wgroup(wd, g + 1)
                    if prefetch is not None and t == HT // 2:
                        prefetch()
                    w = wgroups[g][:, tl * CS:(tl + 1) * CS]
                    rhs = hstrip(t)
                    for mc in range(MC):
                        for qc in range(NQ):
                            nc.tensor.matmul(
                                psq[mc * NQ + qc],
                                w[:, mc * 128:(mc + 1) * 128],
                                rhs[:, qc * 512:(qc + 1) * 512],
                                start=(t == 0), stop=(t == HT - 1),
                            )
                for mc in range(MC):
                    for qc in range(NQ):
                        sink(mc, qc, psq[mc * NQ + qc])

            ecnt = [0]

            def copy3(out_ap, ps):
                # psum->sbuf copies alternate ACT/DVE (gpsimd cannot read
                # PSUM — neuronxcc rejects it)
                eng = (nc.scalar.copy, nc.vector.tensor_copy)[ecnt[0] % 2]
                ecnt[0] += 1
                eng(out_ap, ps)

            def sink_proj(dst):
                def s(mc, qc, ps):
                    copy3(dst[mc][:, qc * 512:(qc + 1) * 512], ps)
                return s

            def sink_out(odram):
                def s(mc, qc, ps):
                    ot = opool.tile([128, 512], F16, tag="ot", name="ot")
                    copy3(ot, ps)
                    nc.gpsimd.dma_start(
                        out=odram[mc * 128:(mc + 1) * 128,
                                  qc * 512:(qc + 1) * 512],
                        in_=ot,
                    )
                return s

            # --- qp pass; prefetch wfk groups 0/1 halfway through ---
            wfk_groups = [None, None, None, None]

            def pf_wfk():
                wfk_groups[0] = load_wgroup(wfk, 0)
                wfk_groups[1] = load_wgroup(wfk, 1)

            proj_pass(wfq_groups, wfq, sink_proj(qpt), prefetch=pf_wfk)

            # --- kp pass; prefetch resident wq (2 MB) halfway through ---
            wq_res = [None]

            def pf_wq():
                wr = wrpool.tile([128, HT * CS], F32R, tag="wres", name="wqr")
                nc.sync.dma_start(out=wr, in_=wq[:, :])
                wq_res[0] = wr

            proj_pass(wfk_groups, wfk, sink_proj(kpt), prefetch=pf_wq)

            # --- scores x wq interleave ---
            # scores: per qt, 8 matmuls into 4 banks (s0..s3); the relu
            # reduction drains on ACT (tiles 0/1) + DVE (tiles 2/3).  The
            # drain rate (~2.5us/qt) exceeds the PE's 1.8us/qt, so wq
            # matmul chains (single-bank, 16-deep accumulation per
            # (mc,qc)) are interleaved to keep the PE busy.
            relmat = rmpool.tile([128, QT], F32, name="relmat")

            def score_qt(qt):
                spss = [
                    pspool.tile([128, 512], F32, tag=f"s{kc}", name="sps")
                    for kc in range(NQ)
                ]
                for dcc in range(DC):
                    for kc in range(NQ):
                        nc.tensor.matmul(
                            spss[kc],
                            qpt[dcc][:, qt * 128:(qt + 1) * 128],
                            kpt[dcc][:, kc * 512:(kc + 1) * 512],
                            start=(dcc == 0), stop=(dcc == DC - 1),
                        )
                relcols = rcpool.tile([128, NQ + 1], F32, tag="relcols",
                                      name="relcols")
                for kc in (0, 1):
                    scratch = scpool.tile([128, 512], F16, tag=f"scr{kc}",
                                          name="scratch")
                    nc.scalar.activation(
                        scratch, spss[kc], AF.Relu,
                        accum_out=relcols[:, kc:kc + 1],
                    )
                wide = scpool.tile([128, 1024], F16, tag="scrw", name="scrw")
                nc.vector.tensor_scalar_max(wide[:, 0:512], spss[2], 0.0)
                nc.vector.tensor_scalar_max(wide[:, 512:1024], spss[3], 0.0)
                nc.vector.tensor_reduce(
                    relcols[:, 2:3], wide,
                    axis=mybir.AxisListType.X, op=mybir.AluOpType.add,
                )
                relscr = rcpool.tile([128, 3], F32, tag="relscr",
                                     name="relscr")
                nc.scalar.activation(
                    relscr, relcols[:, 0:3], AF.Copy,
                    accum_out=relmat[:, qt:qt + 1],
                )

            # wq interleave: 8 chains of (mc,qc), each 16 accumulating
            # matmuls into one rotating bank (m0..m3).
            wq_sink = sink_out(qT)
            wq_chains = [(mc, qc) for mc in range(MC) for qc in range(NQ)]
            wq_state = {"chain": 0, "t": 0, "ps": None}

            def wq_step(nmm):
                # emit nmm wq matmuls (advancing chain state)
                wr = wq_res[0]
                for _ in range(nmm):
                    ci = wq_state["chain"]
                    if ci >= len(wq_chains):
                        return
                    mc, qc = wq_chains[ci]
                    t = wq_state["t"]
                    if t == 0:
                        wq_state["ps"] = pspool.tile(
                            [128, 512], F32, tag=f"m{ci % 4}", name="wqps")
                    nc.tensor.matmul(
                        wq_state["ps"],
                        wr[:, t * CS + mc * 128:t * CS + mc * 128 + 128],
                        hstrip(t)[:, qc * 512:(qc + 1) * 512],
                        start=(t == 0), stop=(t == HT - 1),
                    )
                    if t == HT - 1:
                        wq_sink(mc, qc, wq_state["ps"])
                        wq_state["chain"] = ci + 1
                        wq_state["t"] = 0
                    else:
                        wq_state["t"] = t + 1

            wk_groups = [None, None, None, None]

            def pf_wk():
                wk_groups[0] = load_wgroup(wk, 0)
                wk_groups[1] = load_wgroup(wk, 1)

            for qt in range(QT):
                score_qt(qt)
                if qt == QT - 4:
                    pf_wk()
                wq_step(7)  # 16 qt x 7 ~= 128 posted; rest drained after
            wq_step(128)
            nc.sync.dma_start(out=rel[:, :], in_=relmat)

            # --- wk / wv full 8-bank passes ---
            wv_groups = [None, None, None, None]

            def pf_wv():
                wv_groups[0] = load_wgroup(wv, 0)
                wv_groups[1] = load_wgroup(wv, 1)

            proj_pass(wk_groups, wk, sink_out(kT), prefetch=pf_wv)
            proj_pass(wv_groups, wv, sink_out(vT))
    nc.compile()
    return nc


def build_attn(S=2048, H=HIDDEN, NHC=NUM_HEADS // N_CORES, HD=HEAD_DIM,
               window=LOCAL_WINDOW):
    """Launch B: per-core (2 heads) masked softmax attention + out-proj,
    with the out-proj interleaved into head 1's attention."""
    nc = bacc.Bacc("TRN2", target_bir_lowering=False, debug=False)
    KC, NQ, QT, OCC = S // 128, S // 512, S // 128, H // 512
    qTh = nc.dram_tensor("qTh", [NHC * HD, S], F16, kind="ExternalInput")
    kTh = nc.dram_tensor("kTh", [NHC * HD, S], F16, kind="ExternalInput")
    vTh = nc.dram_tensor("vTh", [NHC * HD, S], F16, kind="ExternalInput")
    woh = nc.dram_tensor("woh", [NHC * HD, H], F16, kind="ExternalInput")
    iotar = nc.dram_tensor("iotar", [128, S], F16, kind="ExternalInput")
    hivec = nc.dram_tensor("hivec", [S], F16, kind="ExternalInput")
    selv = nc.dram_tensor("selv", [S], F16, kind="ExternalInput")
    onesrow = nc.dram_tensor("onesrow", [128], F32R, kind="ExternalInput")
    part = nc.dram_tensor("part", [S, H], F16, kind="ExternalOutput")

    scale = 1.0 / math.sqrt(HD)
    AF = mybir.ActivationFunctionType
    OP = mybir.AluOpType
    VSL_KC = 8  # far blocks only exist for kc <= 7

    with TileContext(nc) as tc:
        with (
            tc.tile_pool(name="const", bufs=1) as cpool,
            tc.tile_pool(name="qk", bufs=1) as qkpool,
            tc.tile_pool(name="vt", bufs=1) as vtpool,
            tc.tile_pool(name="vh", bufs=1) as vhpool,
            tc.tile_pool(name="vsl", bufs=1) as vslpool,
            tc.tile_pool(name="et", bufs=3) as etpool,
            tc.tile_pool(name="aon", bufs=1) as aopool,
            tc.tile_pool(name="dr", bufs=2) as drpool,
            tc.tile_pool(name="ev", bufs=4) as evpool,
            tc.tile_pool(name="ps", bufs=1, space="PSUM") as pspool,
        ):
            # DMA priority: head-0 q/k first (gates first matmul), then
            # v0, consts, head-1 tensors, wo.
            qsb, ksb, vts = [None, None], [None, None], [None, None]
            for h in range(NHC):
                qsb[h] = qkpool.tile([128, S], F16, name=f"qsb{h}")
                nc.sync.dma_start(out=qsb[h], in_=qTh[h * HD:(h + 1) * HD, :])
                ksb[h] = qkpool.tile([128, S], F16, name=f"ksb{h}")
                nc.sync.dma_start(out=ksb[h], in_=kTh[h * HD:(h + 1) * HD, :])
                if h == 0:
                    vts[0] = vtpool.tile([128, S], F16, name="vts0")
                    nc.sync.dma_start(out=vts[0], in_=vTh[0:HD, :])
                    ident = cpool.tile([128, 128], F16, name="ident")
                    make_identity(nc, ident)
                    iota = cpool.tile([128, S], F16, name="iota")
                    nc.sync.dma_start(out=iota, in_=iotar[:, :])
                    ones = cpool.tile([128, 1], F16, name="ones")
                    nc.vector.memset(ones, 1.0)
                    hvec = cpool.tile([128, KC], F16, name="hvec")
                    nc.sync.dma_start(
                        out=hvec, in_=hivec.rearrange("(t p) -> p t", p=128))
                    svec = cpool.tile([128, KC], F16, name="svec")
                    nc.sync.dma_start(
                        out=svec, in_=selv.rearrange("(t p) -> p t", p=128))
                    svec32 = cpool.tile([128, KC], F32, name="svec32")
                    nc.vector.tensor_copy(svec32, svec)
                    ones1 = cpool.tile([1, 128], F32R, name="ones1")
                    nc.sync.dma_start(out=ones1, in_=onesrow[None, :])
            vts[1] = vtpool.tile([128, S], F16, name="vts1")
            nc.sync.dma_start(out=vts[1], in_=vTh[HD:2 * HD, :])
            wsb = []
            for h in range(NHC):
                w = qkpool.tile([128, H], F16, name=f"wsb{h}")
                nc.sync.dma_start(out=w, in_=woh[h * HD:(h + 1) * HD, :])
                wsb.append(w)

            aon = [aopool.tile([128, S], F16, name=f"aon{h}") for h in range(NHC)]
            vhf = [vhpool.tile([128, S], F16, name=f"vhf{h}") for h in range(NHC)]
            vsl = [vslpool.tile([128, VSL_KC * 128], F16, name=f"vsl{h}")
                   for h in range(NHC)]

            ecnt = [0]

            def evict(out_ap, ps):
                eng = (nc.scalar.copy, nc.vector.tensor_copy)[ecnt[0] % 2]
                ecnt[0] += 1
                eng(out_ap, ps)

            vprep_done = [[False] * KC, [False] * KC]

            def vprep(h, kc):
                # lazy per-kc v transpose (+ sel-premult for far-capable kc)
                if kc >= KC or vprep_done[h][kc]:
                    return
                vprep_done[h][kc] = True
                tp = pspool.tile([128, 128], F16, tag="sc", bufs=3, name="tp")
                nc.tensor.transpose(tp, vts[h][:, kc * 128:(kc + 1) * 128], ident)
                dst = vhf[h][:, kc * 128:(kc + 1) * 128]
                evict(dst, tp)
                if kc < VSL_KC:
                    nc.vector.tensor_scalar_mul(
                        vsl[h][:, kc * 128:(kc + 1) * 128], dst,
                        svec32[:, kc:kc + 1],
                    )

            # out-proj slot machine: once head 1's softmax chain for qc is
            # done, its 16 (qt, oc) slots become pending; op_step() emits a
            # couple at a time between attention iterations so the
            # eviction-gated slots never serialize the in-order PE queue.
            # Banks: rotate over the freed av tags.
            op_pending = []
            op_tags = []
            op_n = [0]

            def op_enq(qc):
                op_tags.append(f"av{qc}")
                for qt in range(qc * 4, qc * 4 + 4):
                    for oc in range(OCC):
                        op_pending.append((qt, oc))

            def op_step(n):
                for _ in range(n):
                    if not op_pending:
                        return
                    qt, oc = op_pending.pop(0)
                    ps = pspool.tile([128, 512], F32,
                                     tag=op_tags[op_n[0] % len(op_tags)],
                                     name="wops")
                    op_n[0] += 1
                    for h in range(NHC):
                        nc.tensor.matmul(
                            ps, aon[h][:, qt * 128:(qt + 1) * 128],
                            wsb[h][:, oc * 512:(oc + 1) * 512],
                            start=(h == 0), stop=(h == NHC - 1),
                        )
                    ot = evpool.tile([128, 512], F16, tag="ot", name="ot")
                    evict(ot, ps)
                    nc.sync.dma_start(
                        out=part[qt * 128:(qt + 1) * 128,
                                 oc * 512:(oc + 1) * 512],
                        in_=ot,
                    )

            from collections import deque
            pend = deque()
            for h in range(NHC):
                vprep(h, 0)
                vprep(h, 1)
                avp = [
                    pspool.tile([128, 512], F32, tag=f"av{qc}", bufs=1,
                                name=f"av{qc}")
                    for qc in range(NQ)
                ]
                den128 = pspool.tile([128, 512], F32, tag="den", bufs=1,
                                     name="den128")

                def chain(qc, h=h, avp=avp, den128=den128):
                    # normalize qc: den broadcast via PE (one short ACT hop),
                    # reciprocal + multiply on DVE.
                    q0 = qc * 512
                    dq = drpool.tile([1, 512], F32R, tag=f"dq{qc}",
                                     name=f"dq{qc}")
                    nc.scalar.copy(dq, den128[32 * qc:32 * qc + 1, :])
                    rb = pspool.tile([128, 512], F32, tag="sc", bufs=3,
                                     name="rb")
                    nc.tensor.matmul(rb, ones1, dq, start=True, stop=True)
                    rbs = drpool.tile([128, 512], F32, tag="rbs", name="rbs")
                    rs = drpool.tile([128, 512], F32, tag="rs", name="rs")
                    nc.vector.reciprocal_approx_accurate(rbs, rb, rs)
                    nc.vector.scalar_tensor_tensor(
                        aon[h][:, q0:q0 + 512], rbs, 1.0, avp[qc],
                        op0=OP.mult, op1=OP.mult,
                    )
                    if h == NHC - 1:
                        op_enq(qc)

                def av_den(kc, qcs, far, ets, h=h, avp=avp, den128=den128,
                           chain=chain):
                    for qc in qcs:
                        lhs_av = (vsl[h][:, kc * 128:(kc + 1) * 128]
                                  if far[qc] else
                                  vhf[h][:, kc * 128:(kc + 1) * 128])
                        nc.tensor.matmul(
                            avp[qc], lhs_av, ets[qc],
                            start=(kc == 0), stop=(kc == (qc * 512 + 511) // 128),
                        )
                    for qc in qcs:
                        stop_kc = (qc * 512 + 511) // 128
                        lhs_den = svec[:, kc:kc + 1] if far[qc] else ones
                        nc.tensor.matmul(
                            den128[32 * qc:32 * qc + 1, :], lhs_den, ets[qc],
                            start=(kc == 0), stop=(kc == stop_kc),
                            tile_position=(0, 32 * qc),
                        )
                        if kc == stop_kc:
                            chain(qc)

                # software-pipelined by two kc stages; the deque spans
                # the head boundary so head 1's independent qk/exp work
                # fills head 0's thin-tail PE bubbles
                for kc in range(KC):
                    vprep(h, kc + 2)
                    k0 = kc * 128
                    qcs = [qc for qc in range(NQ) if k0 <= qc * 512 + 511]
                    far = {qc: qc * 512 > k0 + 127 + window for qc in qcs}
                    ets = {}
                    for qc in qcs:
                        q0 = qc * 512
                        q1 = q0 + 511
                        sps = pspool.tile([128, 512], F32, tag="sc", bufs=3,
                                          name="sps")
                        nc.tensor.matmul(
                            sps, ksb[h][:, kc * 128:(kc + 1) * 128],
                            qsb[h][:, q0:q0 + 512], start=True, stop=True,
                        )
                        et = etpool.tile([128, 512], F16, tag=f"et{qc}",
                                         bufs=4, name=f"et{qc}")
                        ets[qc] = et
                        nc.scalar.activation(et, sps, AF.Exp, scale=scale)
                        if far[qc]:
                            continue  # sel-mask folded into vsl/svec operands
                        if q0 < k0 + 128:
                            # causal: zero where q < k (iota - k < 0)
                            nc.gpsimd.affine_select(
                                out=et, in_=et, compare_op=OP.is_ge, fill=0.0,
                                base=q0 - k0, channel_multiplier=-1,
                                pattern=[[1, 512]],
                            )
                        if q1 > k0 + window:
                            nc.vector.scalar_tensor_tensor(
                                et, iota[:, q0:q0 + 512], hvec[:, kc:kc + 1], et,
                                op0=OP.is_le, op1=OP.mult,
                            )
                    pend.append((av_den, (kc, qcs, far, ets)))
                    # h0's last iterations are stall-prone (thin kc tail);
                    # defer their av/den into h1's work-rich start, where
                    # their ACT/DVE chains have already completed
                    maxd = 5 if (h == 0 and kc >= KC - 3) else 2
                    while len(pend) > maxd:
                        fn, args = pend.popleft()
                        fn(*args)
                        op_step(2)
            # final drain: flood the PE queue with ready out-proj slots
            # before and between the last chain-stalled pops — their
            # ACT/DVE chains complete while the op slots execute
            op_step(10)
            while pend:
                fn, args = pend.popleft()
                fn(*args)
                op_step(6)
            while op_pending:
                op_step(4)
    nc.compile()
    return nc


_CACHE = {}


def _get(name, builder, *args):
    key = (name,) + args
    if key not in _CACHE:
        _CACHE[key] = builder(*args)
    return _CACHE[key]


def _run(nc, in_maps):
    res = run_bass_kernel_spmd(
        nc, in_maps, core_ids=list(range(N_CORES)), trace=_TRACE["on"]
    )
    if _TRACE["on"] and res.exec_time_ns is not None:
        _TRACE["exec_ns"].append(res.exec_time_ns)
    return res.results


def kernel(hidden_states, Wq, Wk, Wv, Wo, Wq_ind, Wk_ind, head_weights,
           temperature_param):
    hidden_states = np.asarray(hidden_states, dtype=FP32)
    Wq, Wk, Wv, Wo = (np.asarray(a, dtype=FP32) for a in (Wq, Wk, Wv, Wo))
    Wq_ind = np.asarray(Wq_ind, dtype=FP32)
    Wk_ind = np.asarray(Wk_ind, dtype=FP32)
    head_weights = np.asarray(head_weights, dtype=FP32)

    B, S, H = hidden_states.shape
    assert B == 1 and H == HIDDEN
    CS = H // N_CORES
    D = IND_DIM
    HT = H // 128
    Wfq = Wq @ Wq_ind  # fused indexer weights (f32 host fuse)
    Wfk = Wk @ Wk_ind

    def pmajor(x):
        # (H, C) -> (128, HT*C): out[p, t*C+c] = x[t*128+p, c]
        C = x.shape[1]
        return np.ascontiguousarray(
            x.reshape(HT, 128, C).transpose(1, 0, 2).reshape(128, HT * C))

    hidT = pmajor(np.ascontiguousarray(hidden_states[0].T))

    # ---- Launch A: projections + indexer, head-parallel ----
    ncA = _get("A", build_fused, S, H, CS, D)
    inA = [
        {
            "hidT": hidT,
            "wq": pmajor(Wq[:, c * CS:(c + 1) * CS]),
            "wk": pmajor(Wk[:, c * CS:(c + 1) * CS]),
            "wv": pmajor(Wv[:, c * CS:(c + 1) * CS]),
            "wfq": pmajor(Wfq[:, c * D:(c + 1) * D]),
            "wfk": pmajor(Wfk[:, c * D:(c + 1) * D]),
        }
        for c in range(N_CORES)
    ]
    rA = _run(ncA, inA)
    rel = np.zeros(S, dtype=np.float64)
    for c in range(N_CORES):
        # rel arrives as relmat [128, QT]: rel[t*128+p] = relmat[p, t]
        rel += float(head_weights[c]) * \
            rA[c]["rel"].astype(np.float64).T.ravel()
    # exp(-temp) scaling is monotone; irrelevant for top-k selection.

    k_sel = min(MAX_SELECTED, S)
    top_idx = np.argpartition(-rel, k_sel - 1)[:k_sel]
    selected = np.zeros(S, dtype=bool)
    selected[top_idx] = True

    # ---- Launch B: masked attention + output projection, head-parallel ----
    BIG = float(2 * S + 1024)
    hi = np.where(selected, BIG, np.arange(S, dtype=np.float64) + LOCAL_WINDOW)
    hi = hi.astype(np.float16)
    selv = selected.astype(np.float16)
    iotar = np.broadcast_to(
        np.arange(S, dtype=np.float16)[None, :], (128, S)).copy()
    NHC = NUM_HEADS // N_CORES
    RW = NHC * HEAD_DIM
    ncB = _get("B", build_attn, S, H, NHC, HEAD_DIM, LOCAL_WINDOW)
    inB = [
        {
            "qTh": rA[c]["qT"],
            "kTh": rA[c]["kT"],
            "vTh": rA[c]["vT"],
            "woh": np.ascontiguousarray(Wo[c * RW:(c + 1) * RW]).astype(
                np.float16),
            "iotar": iotar,
            "hivec": hi,
            "selv": selv,
            "onesrow": np.ones(128, dtype=np.float32),
        }
        for c in range(N_CORES)
    ]
    rB = _run(ncB, inB)
    out = rB[0]["part"].astype(np.float32)
    for c in range(1, N_CORES):
        out += rB[c]["part"].astype(np.float32)
    return out.reshape(B, S, H)
